# revision 11
# baseline (speedup 1.0000x reference)
"""Multi-head attention forward (B=4, N=2048, C=1024, H=16) on 8 TRN2 NeuronCores.

Sharding: 8 shards = (batch b, query-half). Each core computes Q for its 1024
query tokens and K/V for the full 2048 tokens of its batch (K/V projection
duplicated across the 2 cores sharing a batch — cheaper than communicating),
then attention + output projection for its queries. Zero collectives.

bf16 TensorEngine compute, f32 PSUM accumulation. Scores computed transposed
(ST[keys, q]) so softmax needs no transposes: exp on the ScalarEngine (no max
subtraction — scores are bounded), denominator via a ones-column appended to
V, 1/den via reciprocal_approx_fast + stride-0 DMA partition-broadcast.
Q/K projections for head-pair pr+2 are interleaved between attention blocks
so projection matmuls fill the ACT-bound PE gaps (keeps HAM at full clock).
"""

from contextlib import ExitStack

import numpy as np
import ml_dtypes

import concourse.bass as bass
import concourse.bacc as bacc
import concourse.tile as tile
import concourse.mybir as mybir
from concourse.bass_utils import run_bass_kernel_spmd

F32 = mybir.dt.float32
BF16 = mybir.dt.bfloat16
AF = mybir.ActivationFunctionType
ALU = mybir.AluOpType
BF = ml_dtypes.bfloat16

P = 128
D = 1024
CC = 8
H = 16
DH = 64
NKV = 2048
NQ = 1024
TB = NKV // P
KC = NKV // P
SCALE = DH ** -0.5
VS = 2 * (8 * 65 + 64)  # v slab: two 8-head groups, each padded 64 cols so attnV lhsT [128,128] (FWL) never reads across groups


def attention_body(tc, out, xT, wqT, wkT, wvT, woT, bq, bk, bv, bo):
    nc = tc.nc
    with ExitStack() as ctx:
        const = ctx.enter_context(tc.tile_pool(name="const", bufs=1))
        qkv = ctx.enter_context(tc.tile_pool(name="qkv", bufs=1))
        xw = ctx.enter_context(tc.tile_pool(name="xw", bufs=1))
        wst = ctx.enter_context(tc.tile_pool(name="wst", bufs=2))
        wib = ctx.enter_context(tc.tile_pool(name="wib", bufs=3))
        ee = ctx.enter_context(tc.tile_pool(name="ee", bufs=3))
        rc = ctx.enter_context(tc.tile_pool(name="rc", bufs=1))
        fo = ctx.enter_context(tc.tile_pool(name="fo", bufs=2))
        sp = ctx.enter_context(tc.tile_pool(name="sp", bufs=2, space="PSUM"))
        ao = ctx.enter_context(tc.tile_pool(name="ao", bufs=1, space="PSUM"))
        pj = ctx.enter_context(tc.tile_pool(name="pj", bufs=2, space="PSUM"))

        bq_sb = const.tile([P, CC], F32)
        bk_sb = const.tile([P, CC], F32)
        bv_sb = const.tile([P, CC], F32)
        bo_sb = const.tile([1, D], F32)
        nc.sync.dma_start(bq_sb[:, :], bq[:, :])
        nc.sync.dma_start(bk_sb[:, :], bk[:, :])
        nc.sync.dma_start(bv_sb[:, :], bv[:, :])
        nc.sync.dma_start(bo_sb[:, :], bo[:, :])
        onesf = const.tile([1, P], F32)
        nc.vector.memset(onesf[:, :], 1.0)
        # selector weights for the 1/den partition-broadcast matmuls:
        # selA -> output partitions 0:64 (head A), selB -> 64:128 (head B)
        selA = const.tile([1, P], BF16)
        nc.vector.memset(selA[0:1, 0:64], 1.0)
        nc.vector.memset(selA[0:1, 64:128], 0.0)
        selB = const.tile([1, P], BF16)
        nc.vector.memset(selB[0:1, 0:64], 0.0)
        nc.vector.memset(selB[0:1, 64:128], 1.0)
        bo_bc = const.tile([P, D], BF16)

        qT_sb = qkv.tile([P, CC * NQ], BF16)
        kT_sb = qkv.tile([P, CC * NKV], BF16)
        v_sb = qkv.tile([P, TB * VS], BF16)   # per-tb slab: 16*65 + 64 pad
        yT_sb = qkv.tile([P, CC * NQ], BF16)

        def load_w(wT_dram):
            w_sb = wst.tile([P, CC * D], BF16, tag="w")
            for cc in range(CC):
                nc.sync.dma_start(w_sb[:, cc * D:(cc + 1) * D], wT_dram[cc * P:(cc + 1) * P, :])
            return w_sb

        wv_sb = load_w(wvT)   # slot 0 (slot 1 goes to wo later)

        # xT loaded in 512-token blocks so v_proj(0..3) can start after the
        # first ~1MB instead of the full 4MB
        xT_sb = xw.tile([P, CC * NKV], BF16)
        for blk in range(4):
            for cc in range(CC):
                nc.sync.dma_start(
                    xT_sb[:, cc * NKV + blk * 512: cc * NKV + (blk + 1) * 512],
                    xT[cc * P:(cc + 1) * P, blk * 512:(blk + 1) * 512])

        def load_w_ib(wT_dram, ib):
            """JIT [1024, 128] column-slice of a weight matrix for one i-block."""
            w_sb = wib.tile([P, CC * P], BF16, tag="wib")
            for cc in range(CC):
                nc.sync.dma_start(
                    w_sb[:, cc * P:(cc + 1) * P],
                    wT_dram[cc * P:(cc + 1) * P, ib * P:(ib + 1) * P])
            return w_sb

        v4 = v_sb.rearrange("p (t g s) -> p t g s", t=TB, g=2)
        nc.vector.memset(v4[:, :, :, 8 * 65:], 0.0)
        v5 = v4[:, :, :, 0:8 * 65].rearrange("p t g (h c) -> p t g h c", c=65)
        nc.vector.memset(v5[:, :, :, :, 64:65], 1.0)

        def v_proj(tb, nch):
            ps = pj.tile([P, 512], F32, tag="ps")
            for cc in range(CC):
                nc.tensor.matmul(
                    ps[:, :],
                    xT_sb[:, cc * NKV + tb * P: cc * NKV + (tb + 1) * P],
                    wv_sb[:, cc * D + nch * 512: cc * D + nch * 512 + 512],
                    start=(cc == 0), stop=(cc == CC - 1))
            vsrc = ps.rearrange("p (h c) -> p h c", c=64)
            base = tb * VS + nch * (8 * 65 + 64)
            dst = v_sb[:, base: base + 8 * 65].rearrange(
                "p (h c) -> p h c", c=65)[:, :, 0:64]
            nc.vector.tensor_copy(dst, vsrc)

        # ---- prologue: V (first head group) ----
        for tb in range(TB):
            v_proj(tb, 0)

        # bo broadcast to all partitions (ones matmul, once)
        for ch in range(2):
            ps = pj.tile([P, 512], F32, tag="ps")
            nc.tensor.matmul(ps[:, :], onesf[:, :], bo_sb[:, ch * 512:(ch + 1) * 512],
                             start=True, stop=True)
            nc.vector.tensor_copy(bo_bc[:, ch * 512:(ch + 1) * 512], ps[:, :])

        def q_proj(ib, w_sb=None):
            if w_sb is None:
                w_sb = load_w_ib(wqT, ib)
            for t in range(NQ // 512):
                ps = pj.tile([P, 512], F32, tag="ps")
                for cc in range(CC):
                    nc.tensor.matmul(
                        ps[:, :],
                        w_sb[:, cc * P:(cc + 1) * P],
                        xT_sb[:, cc * NKV + t * 512: cc * NKV + t * 512 + 512],
                        start=(cc == 0), stop=(cc == CC - 1))
                nc.vector.tensor_scalar(
                    qT_sb[:, ib * NQ + t * 512: ib * NQ + t * 512 + 512],
                    ps[:, :], bq_sb[:, ib:ib + 1], None, op0=ALU.add)

        def k_proj(ib, w_sb=None):
            if w_sb is None:
                w_sb = load_w_ib(wkT, ib)
            for t in range(NKV // 512):
                ps = pj.tile([P, 512], F32, tag="ps")
                for cc in range(CC):
                    nc.tensor.matmul(
                        ps[:, :],
                        w_sb[:, cc * P:(cc + 1) * P],
                        xT_sb[:, cc * NKV + t * 512: cc * NKV + t * 512 + 512],
                        start=(cc == 0), stop=(cc == CC - 1))
                nc.vector.tensor_scalar(
                    kT_sb[:, ib * NKV + t * 512: ib * NKV + t * 512 + 512],
                    ps[:, :], bk_sb[:, ib:ib + 1], None, op0=ALU.add)

        q_proj(0)
        k_proj(0)

        wo_sb = None

        def fin_proj(tb):
            for nch in range(2):
                ps = pj.tile([P, 512], F32, tag="ps")
                for cc in range(CC):
                    nc.tensor.matmul(
                        ps[:, :],
                        yT_sb[:, cc * NQ + tb * P: cc * NQ + (tb + 1) * P],
                        wo_sb[:, cc * D + nch * 512: cc * D + nch * 512 + 512],
                        start=(cc == 0), stop=(cc == CC - 1))
                os = fo.tile([P, 512], F32, tag="o")
                nc.vector.tensor_tensor(os[:, :], ps[:, :],
                                        bo_bc[:, nch * 512:(nch + 1) * 512], op=ALU.add)
                nc.sync.dma_start(out[tb * P:(tb + 1) * P, nch * 512:(nch + 1) * 512],
                                  os[:, :])

        # ---- main: attention per head pair, Q/K for pr+2 woven between ----
        for pr in range(CC):
            hA, hB = 2 * pr, 2 * pr + 1
            for qb in range(NQ // 512):
                qc = qb * 512
                oA = ao.tile([P, 512], F32, tag="oA")
                oB = ao.tile([P, 512], F32, tag="oB")
                vbA = (hA // 8) * 584 + (hA % 8) * 65
                vbB = (hB // 8) * 584 + (hB % 8) * 65
                for kc in range(KC):
                    # one 2-bank score tile per key chunk: [A | B]. A/B are
                    # adjacent matmuls in different PE row bands (64x128
                    # tiling) so they overlap in the array; one exp covers
                    # both; double-buffered so the next chunk's scores run
                    # while this one's exp drains.
                    s2 = sp.tile([P, 1024], F32, tag="s")
                    nc.tensor.matmul(
                        s2[:, 0:512],
                        kT_sb[0:64, pr * NKV + kc * P: pr * NKV + (kc + 1) * P],
                        qT_sb[0:64, pr * NQ + qc: pr * NQ + qc + 512],
                        start=True, stop=True)
                    nc.tensor.matmul(
                        s2[:, 512:1024],
                        kT_sb[64:128, pr * NKV + kc * P: pr * NKV + (kc + 1) * P],
                        qT_sb[64:128, pr * NQ + qc: pr * NQ + qc + 512],
                        start=True, stop=True)
                    e2 = ee.tile([P, 1024], BF16, tag="e")
                    nc.scalar.activation(e2[:, :], s2[:, :], AF.Exp, scale=SCALE)
                    nc.tensor.matmul(
                        oA[:, :],
                        v_sb[:, kc * VS + vbA: kc * VS + vbA + 128],
                        e2[:, 0:512],
                        start=(kc == 0), stop=(kc == KC - 1))
                    nc.tensor.matmul(
                        oB[:, :],
                        v_sb[:, kc * VS + vbB: kc * VS + vbB + 128],
                        e2[:, 512:1024],
                        start=(kc == 0), stop=(kc == KC - 1))
                # evict unnormalized rows first (frees oA/oB banks fast), then
                # normalize yT in place. 1/den broadcast to 64 partitions per
                # head via two rank-1 PE matmuls (ones ⊗ rec) — a stride-0
                # SBUF→SBUF DMA broadcast serializes ~128 descriptors on the
                # source partition's port (~9µs, exposed at the final block)
                yA = yT_sb[0:64, pr * NQ + qc: pr * NQ + qc + 512]
                yB = yT_sb[64:128, pr * NQ + qc: pr * NQ + qc + 512]
                den2 = rc.tile([1, 1024], F32, tag="d")
                nc.vector.tensor_copy(den2[0:1, 0:512], oA[64:65, :])
                nc.vector.tensor_copy(yA, oA[0:64, :])
                nc.vector.tensor_copy(den2[0:1, 512:1024], oB[64:65, :])
                nc.vector.tensor_copy(yB, oB[0:64, :])
                rec2 = rc.tile([1, 1024], F32, tag="rf")
                nc.vector.reciprocal_approx_fast(rec2[0:1, :], den2[0:1, :])
                rec2b = rc.tile([1, 1024], BF16, tag="rb")
                nc.vector.tensor_copy(rec2b[0:1, :], rec2[0:1, :])
                bc2 = pj.tile([P, 512], F32, tag="ps")
                nc.tensor.matmul(bc2[:, :], selA[0:1, :], rec2b[0:1, 0:512],
                                 start=True, stop=False)
                nc.tensor.matmul(bc2[:, :], selB[0:1, :], rec2b[0:1, 512:1024],
                                 start=False, stop=True)
                nc.vector.tensor_tensor(yA, yA, bc2[0:64, :], op=ALU.mult)
                nc.vector.tensor_scalar(yA, yA, bv_sb[0:64, pr:pr + 1], None, op0=ALU.add)
                nc.vector.tensor_tensor(yB, yB, bc2[64:128, :], op=ALU.mult)
                nc.vector.tensor_scalar(yB, yB, bv_sb[64:128, pr:pr + 1], None, op0=ALU.add)

                # weave next projections / wo load into the ACT-bound stretch
                if pr < 4:
                    v_proj(4 * pr + 2 * qb, 1)       # needed from pr=4 on
                    v_proj(4 * pr + 2 * qb + 1, 1)
                if qb == 0 and pr + 1 < CC:
                    # issue both weight-slice DMAs now so the K slice has a
                    # full half-slot to land before its matmuls need it
                    wq_next = load_w_ib(wqT, pr + 1)
                    wk_next = load_w_ib(wkT, pr + 1)
                    q_proj(pr + 1, wq_next)
                elif qb == 1 and pr + 1 < CC:
                    k_proj(pr + 1, wk_next)
                if pr == 5 and qb == 1:
                    wo_sb = load_w(woT)   # slot 1 (wv in slot 0 is done by pr=4)
                if pr == CC - 1 and qb == 1:
                    # qb0 output rows are complete once the last head pair's
                    # qb0 eviction lands -- weave their output projection here
                    for ftb in range(NQ // P // 2):
                        fin_proj(ftb)

        # ---- output projection (second token half; first half woven above).
        # The score pool is idle now: run pairs of psum groups through its
        # [128,1024] tiles for 3x the bank throughput of pj.
        for tb in range(NQ // P // 2, NQ // P, 2):
            for nch in range(2):
                ps = sp.tile([P, 1024], F32, tag="s")
                for j in range(2):
                    for cc in range(CC):
                        nc.tensor.matmul(
                            ps[:, j * 512:(j + 1) * 512],
                            yT_sb[:, cc * NQ + (tb + j) * P: cc * NQ + (tb + j + 1) * P],
                            wo_sb[:, cc * D + nch * 512: cc * D + nch * 512 + 512],
                            start=(cc == 0), stop=(cc == CC - 1))
                os = fo.tile([P, 1024], F32, tag="o2")
                nc.vector.tensor_tensor(os[:, 0:512], ps[:, 0:512],
                                        bo_bc[:, nch * 512:(nch + 1) * 512], op=ALU.add)
                nc.vector.tensor_tensor(os[:, 512:1024], ps[:, 512:1024],
                                        bo_bc[:, nch * 512:(nch + 1) * 512], op=ALU.add)
                nc.sync.dma_start(out[tb * P:(tb + 1) * P, nch * 512:(nch + 1) * 512],
                                  os[:, 0:512])
                nc.sync.dma_start(out[(tb + 1) * P:(tb + 2) * P, nch * 512:(nch + 1) * 512],
                                  os[:, 512:1024])


N_CORES = 8


_GRAPH_CACHE = {}


def build_graph():
    if "nc" in _GRAPH_CACHE:
        return _GRAPH_CACHE["nc"]
    nc = bacc.Bacc("TRN2", target_bir_lowering=False, debug=False,
                   num_devices=N_CORES)
    xT = nc.dram_tensor("xT", [D, NKV], BF16, kind="ExternalInput").ap()
    wqT = nc.dram_tensor("wqT", [D, D], BF16, kind="ExternalInput").ap()
    wkT = nc.dram_tensor("wkT", [D, D], BF16, kind="ExternalInput").ap()
    wvT = nc.dram_tensor("wvT", [D, D], BF16, kind="ExternalInput").ap()
    woT = nc.dram_tensor("woT", [D, D], BF16, kind="ExternalInput").ap()
    bq = nc.dram_tensor("bq", [P, CC], F32, kind="ExternalInput").ap()
    bk = nc.dram_tensor("bk", [P, CC], F32, kind="ExternalInput").ap()
    bv = nc.dram_tensor("bv", [P, CC], F32, kind="ExternalInput").ap()
    bo = nc.dram_tensor("bo", [1, D], F32, kind="ExternalInput").ap()
    out = nc.dram_tensor("out", [NQ, D], F32, kind="ExternalOutput").ap()
    with tile.TileContext(nc) as tc:
        attention_body(tc, out, xT, wqT, wkT, wvT, woT, bq, bk, bv, bo)
    nc.compile()
    _GRAPH_CACHE["nc"] = nc
    return nc


def make_in_maps(x, Wq, bq, Wk, bk, Wv, bv, Wo, bo):
    x = np.asarray(x, np.float32)
    shared = {
        "wqT": np.ascontiguousarray(np.asarray(Wq, np.float32).T).astype(BF),
        "wkT": np.ascontiguousarray(np.asarray(Wk, np.float32).T).astype(BF),
        "wvT": np.ascontiguousarray(np.asarray(Wv, np.float32).T).astype(BF),
        "woT": np.ascontiguousarray(np.asarray(Wo, np.float32).T).astype(BF),
        "bq": np.ascontiguousarray(np.asarray(bq, np.float32).reshape(CC, P).T),
        "bk": np.ascontiguousarray(np.asarray(bk, np.float32).reshape(CC, P).T),
        "bv": np.ascontiguousarray(np.asarray(bv, np.float32).reshape(CC, P).T),
        "bo": np.asarray(bo, np.float32).reshape(1, D),
    }
    in_maps = []
    for core in range(N_CORES):
        b, half = core // 2, core % 2
        xb = x[b]
        if half == 1:
            xb = np.concatenate([xb[NQ:], xb[:NQ]], axis=0)
        xT = np.ascontiguousarray(xb.T).astype(BF)
        in_maps.append({"xT": xT, **shared})
    return in_maps


def run(inputs, trace=False, **kw):
    nc = build_graph()
    in_maps = make_in_maps(**inputs)
    res = run_bass_kernel_spmd(nc, in_maps, list(range(N_CORES)), trace=trace, **kw)
    x = np.asarray(inputs["x"], np.float32)
    B, N, C = x.shape
    out = np.empty((B, N, C), np.float32)
    for core in range(N_CORES):
        b, half = core // 2, core % 2
        out[b, half * NQ:(half + 1) * NQ, :] = res.results[core]["out"]
    return out, res


def kernel(x, Wq, bq, Wk, bk, Wv, bv, Wo, bo):
    out, _ = run(dict(x=x, Wq=Wq, bq=bq, Wk=Wk, bk=bk, Wv=Wv, bv=bv, Wo=Wo, bo=bo))
    return out



# revision 13
# speedup vs baseline: 1.0880x; 1.0880x over previous
"""Multi-head attention forward (B=4, N=2048, C=1024, H=16) on 8 TRN2 NeuronCores.

Sharding: 8 shards = (batch b, query-half). Each core computes Q for its 1024
query tokens and K/V for the full 2048 tokens of its batch (K/V projection
duplicated across the 2 cores sharing a batch — cheaper than communicating),
then attention + output projection for its queries. Zero collectives.

bf16 TensorEngine compute, f32 PSUM accumulation. Scores computed transposed
(ST[keys, q]) so softmax needs no transposes: exp on the ScalarEngine (no max
subtraction — scores are bounded). Each head's V slab is [V(64) | ones(64)],
so the attnV matmul lands the softmax denominator replicated on PSUM
partitions 64:127 at no extra cost (same 512-column stream) — normalization
is then a partition-shifted reciprocal+multiply on the VectorEngine with no
cross-partition broadcast. kT/qT live in 3-deep rotating rings (the per-pr
slabs are produced one head-pair ahead) to make room for the wider V slab.
Q/K projections for head-pair pr+1 are interleaved between attention blocks
so projection matmuls fill the exp-bound PE gaps; score/attnV matmuls are
emitted in 2-chunk batches to halve tiled<->full PE mode switches.
"""

from contextlib import ExitStack

import numpy as np
import ml_dtypes

import concourse.bass as bass
import concourse.bacc as bacc
import concourse.tile as tile
import concourse.mybir as mybir
from concourse.bass_utils import run_bass_kernel_spmd

F32 = mybir.dt.float32
BF16 = mybir.dt.bfloat16
AF = mybir.ActivationFunctionType
ALU = mybir.AluOpType
BF = ml_dtypes.bfloat16

P = 128
D = 1024
CC = 8
H = 16
DH = 64
NKV = 2048
NQ = 1024
TB = NKV // P
KC = NKV // P
SCALE = DH ** -0.5
VS = H * P  # v slab per key-chunk: 16 heads x [V 64 | ones 64]


def attention_body(tc, out, xT, wqT, wkT, wvT, woT, bq, bk, bv, bo):
    nc = tc.nc
    with ExitStack() as ctx:
        const = ctx.enter_context(tc.tile_pool(name="const", bufs=1))
        qkv = ctx.enter_context(tc.tile_pool(name="qkv", bufs=1))
        qring = ctx.enter_context(tc.tile_pool(name="qring", bufs=3))
        kring = ctx.enter_context(tc.tile_pool(name="kring", bufs=3))
        xw = ctx.enter_context(tc.tile_pool(name="xw", bufs=1))
        wst = ctx.enter_context(tc.tile_pool(name="wst", bufs=2))
        wib = ctx.enter_context(tc.tile_pool(name="wib", bufs=3))
        ee = ctx.enter_context(tc.tile_pool(name="ee", bufs=3))
        rc = ctx.enter_context(tc.tile_pool(name="rc", bufs=1))
        fo = ctx.enter_context(tc.tile_pool(name="fo", bufs=2))
        sp = ctx.enter_context(tc.tile_pool(name="sp", bufs=2, space="PSUM"))
        ao = ctx.enter_context(tc.tile_pool(name="ao", bufs=1, space="PSUM"))
        pj = ctx.enter_context(tc.tile_pool(name="pj", bufs=2, space="PSUM"))

        bq_sb = const.tile([P, CC], F32)
        bk_sb = const.tile([P, CC], F32)
        bv_sb = const.tile([P, CC], F32)
        bo_sb = const.tile([1, D], F32)
        nc.sync.dma_start(bq_sb[:, :], bq[:, :])
        nc.sync.dma_start(bk_sb[:, :], bk[:, :])
        nc.sync.dma_start(bv_sb[:, :], bv[:, :])
        nc.sync.dma_start(bo_sb[:, :], bo[:, :])
        onesf = const.tile([1, P], F32)
        nc.vector.memset(onesf[:, :], 1.0)
        bo_bc = const.tile([P, D], BF16)

        v_sb = qkv.tile([P, TB * VS], BF16)
        yT_sb = qkv.tile([P, CC * NQ], BF16)

        def load_w(wT_dram):
            w_sb = wst.tile([P, CC * D], BF16, tag="w")
            for cc in range(CC):
                nc.sync.dma_start(w_sb[:, cc * D:(cc + 1) * D], wT_dram[cc * P:(cc + 1) * P, :])
            return w_sb

        wv_sb = load_w(wvT)   # slot 0 (slot 1 goes to wo later)

        # xT loaded in 512-token blocks so v_proj(0..3) can start after the
        # first ~1MB instead of the full 4MB
        xT_sb = xw.tile([P, CC * NKV], BF16)
        for blk in range(4):
            for cc in range(CC):
                nc.sync.dma_start(
                    xT_sb[:, cc * NKV + blk * 512: cc * NKV + (blk + 1) * 512],
                    xT[cc * P:(cc + 1) * P, blk * 512:(blk + 1) * 512])

        def load_w_ib(wT_dram, ib):
            """JIT [1024, 128] column-slice of a weight matrix for one i-block."""
            w_sb = wib.tile([P, CC * P], BF16, tag="wib")
            for cc in range(CC):
                nc.sync.dma_start(
                    w_sb[:, cc * P:(cc + 1) * P],
                    wT_dram[cc * P:(cc + 1) * P, ib * P:(ib + 1) * P])
            return w_sb

        # ones half-blocks: head h's slab cols [h*128+64, h*128+128)
        v4 = v_sb.rearrange("p (t h c) -> p t h c", t=TB, c=P)
        nc.vector.memset(v4[:, :, :, 64:128], 1.0)

        def v_proj(tb, nch):
            ps = pj.tile([P, 512], F32, tag="ps")
            for cc in range(CC):
                nc.tensor.matmul(
                    ps[:, :],
                    xT_sb[:, cc * NKV + tb * P: cc * NKV + (tb + 1) * P],
                    wv_sb[:, cc * D + nch * 512: cc * D + nch * 512 + 512],
                    start=(cc == 0), stop=(cc == CC - 1))
            vsrc = ps.rearrange("p (h c) -> p h c", c=64)
            base = tb * VS + nch * 8 * P
            dst = v_sb[:, base: base + 8 * P].rearrange(
                "p (h c) -> p h c", c=P)[:, :, 0:64]
            nc.vector.tensor_copy(dst, vsrc)

        # ---- prologue: V (first head group) ----
        for tb in range(TB):
            v_proj(tb, 0)

        # bo broadcast to all partitions (ones matmul, once)
        for ch in range(2):
            ps = pj.tile([P, 512], F32, tag="ps")
            nc.tensor.matmul(ps[:, :], onesf[:, :], bo_sb[:, ch * 512:(ch + 1) * 512],
                             start=True, stop=True)
            nc.vector.tensor_copy(bo_bc[:, ch * 512:(ch + 1) * 512], ps[:, :])

        def q_proj(ib, w_sb=None):
            if w_sb is None:
                w_sb = load_w_ib(wqT, ib)
            qt = qring.tile([P, NQ], BF16, tag="q")
            for t in range(NQ // 512):
                ps = pj.tile([P, 512], F32, tag="ps")
                for cc in range(CC):
                    nc.tensor.matmul(
                        ps[:, :],
                        w_sb[:, cc * P:(cc + 1) * P],
                        xT_sb[:, cc * NKV + t * 512: cc * NKV + t * 512 + 512],
                        start=(cc == 0), stop=(cc == CC - 1))
                nc.vector.tensor_scalar(
                    qt[:, t * 512: t * 512 + 512],
                    ps[:, :], bq_sb[:, ib:ib + 1], None, op0=ALU.add)
            return qt

        def k_proj(ib, w_sb=None):
            if w_sb is None:
                w_sb = load_w_ib(wkT, ib)
            kt = kring.tile([P, NKV], BF16, tag="k")
            for t in range(NKV // 512):
                ps = pj.tile([P, 512], F32, tag="ps")
                for cc in range(CC):
                    nc.tensor.matmul(
                        ps[:, :],
                        w_sb[:, cc * P:(cc + 1) * P],
                        xT_sb[:, cc * NKV + t * 512: cc * NKV + t * 512 + 512],
                        start=(cc == 0), stop=(cc == CC - 1))
                nc.vector.tensor_scalar(
                    kt[:, t * 512: t * 512 + 512],
                    ps[:, :], bk_sb[:, ib:ib + 1], None, op0=ALU.add)
            return kt

        qts = {0: q_proj(0)}
        kts = {0: k_proj(0)}

        wo_sb = None

        def fin_proj(tb):
            for nch in range(2):
                ps = pj.tile([P, 512], F32, tag="ps")
                for cc in range(CC):
                    nc.tensor.matmul(
                        ps[:, :],
                        yT_sb[:, cc * NQ + tb * P: cc * NQ + (tb + 1) * P],
                        wo_sb[:, cc * D + nch * 512: cc * D + nch * 512 + 512],
                        start=(cc == 0), stop=(cc == CC - 1))
                os = fo.tile([P, 512], F32, tag="o")
                nc.vector.tensor_tensor(os[:, :], ps[:, :],
                                        bo_bc[:, nch * 512:(nch + 1) * 512], op=ALU.add)
                nc.sync.dma_start(out[tb * P:(tb + 1) * P, nch * 512:(nch + 1) * 512],
                                  os[:, :])

        # ---- main: attention per head pair, Q/K for pr+1 woven between ----
        for pr in range(CC):
            hA, hB = 2 * pr, 2 * pr + 1
            qt, kt = qts[pr], kts[pr]
            for qb in range(NQ // 512):
                qc = qb * 512
                oA = ao.tile([P, 512], F32, tag="oA")
                oB = ao.tile([P, 512], F32, tag="oB")
                vbA = hA * P
                vbB = hB * P
                for kc2 in range(KC // 2):
                    # scores+exp for two key chunks, then both attnV pairs:
                    # batching keeps the PE in 64x128-tiled mode across both
                    # score pairs (one tiled<->full transition per 2 chunks)
                    pair = []
                    for kc in (2 * kc2, 2 * kc2 + 1):
                        s2 = sp.tile([P, 1024], F32, tag="s")
                        nc.tensor.matmul(
                            s2[:, 0:512],
                            kt[0:64, kc * P:(kc + 1) * P],
                            qt[0:64, qc: qc + 512],
                            start=True, stop=True)
                        nc.tensor.matmul(
                            s2[:, 512:1024],
                            kt[64:128, kc * P:(kc + 1) * P],
                            qt[64:128, qc: qc + 512],
                            start=True, stop=True)
                        e2 = ee.tile([P, 1024], BF16, tag="e")
                        nc.scalar.activation(e2[:, :], s2[:, :], AF.Exp, scale=SCALE)
                        pair.append((kc, e2))
                    for kc, e2 in pair:
                        nc.tensor.matmul(
                            oA[:, :],
                            v_sb[:, kc * VS + vbA: kc * VS + vbA + P],
                            e2[:, 0:512],
                            start=(kc == 0), stop=(kc == KC - 1))
                        nc.tensor.matmul(
                            oB[:, :],
                            v_sb[:, kc * VS + vbB: kc * VS + vbB + P],
                            e2[:, 512:1024],
                            start=(kc == 0), stop=(kc == KC - 1))
                # normalize: oX rows 64:127 hold den (replicated by the ones
                # half-block); partition-shifted reciprocal + multiply, then
                # bias add in place — frees oA/oB without any copies/broadcast
                yA = yT_sb[0:64, pr * NQ + qc: pr * NQ + qc + 512]
                yB = yT_sb[64:128, pr * NQ + qc: pr * NQ + qc + 512]
                den2 = rc.tile([P, 1024], F32, tag="d")
                nc.vector.tensor_copy(den2[0:64, 0:512], oA[64:128, :])
                nc.vector.tensor_copy(den2[0:64, 512:1024], oB[64:128, :])
                rec2 = rc.tile([P, 1024], F32, tag="rf")
                nc.vector.reciprocal_approx_fast(rec2[0:64, :], den2[0:64, :])
                nc.vector.tensor_tensor(yA, oA[0:64, :], rec2[0:64, 0:512], op=ALU.mult)
                ytmp = rc.tile([P, 512], BF16, tag="yt")
                nc.vector.tensor_tensor(ytmp[0:64, :], oB[0:64, :],
                                        rec2[0:64, 512:1024], op=ALU.mult)
                nc.vector.tensor_copy(yB, ytmp[0:64, :])
                nc.vector.tensor_scalar(yA, yA, bv_sb[0:64, pr:pr + 1], None, op0=ALU.add)
                nc.vector.tensor_scalar(yB, yB, bv_sb[64:128, pr:pr + 1], None, op0=ALU.add)

                # weave next projections / wo load into the exp-bound stretch
                if pr < 4:
                    v_proj(4 * pr + 2 * qb, 1)       # needed from pr=4 on
                    v_proj(4 * pr + 2 * qb + 1, 1)
                if qb == 0 and pr + 1 < CC:
                    # issue both weight-slice DMAs now so the K slice has a
                    # full half-slot to land before its matmuls need it
                    wq_next = load_w_ib(wqT, pr + 1)
                    wk_next = load_w_ib(wkT, pr + 1)
                    qts[pr + 1] = q_proj(pr + 1, wq_next)
                elif qb == 1 and pr + 1 < CC:
                    kts[pr + 1] = k_proj(pr + 1, wk_next)
                if pr == 5 and qb == 1:
                    wo_sb = load_w(woT)   # slot 1 (wv in slot 0 is done by pr=4)
                if pr == CC - 1 and qb == 1:
                    # qb0 output rows are complete once the last head pair's
                    # qb0 eviction lands -- weave their output projection here
                    for ftb in range(NQ // P // 2):
                        fin_proj(ftb)

        # ---- output projection (second token half; first half woven above).
        # The score pool is idle now: run pairs of psum groups through its
        # [128,1024] tiles for 3x the bank throughput of pj.
        for tb in range(NQ // P // 2, NQ // P, 2):
            for nch in range(2):
                ps = sp.tile([P, 1024], F32, tag="s")
                for j in range(2):
                    for cc in range(CC):
                        nc.tensor.matmul(
                            ps[:, j * 512:(j + 1) * 512],
                            yT_sb[:, cc * NQ + (tb + j) * P: cc * NQ + (tb + j + 1) * P],
                            wo_sb[:, cc * D + nch * 512: cc * D + nch * 512 + 512],
                            start=(cc == 0), stop=(cc == CC - 1))
                os = fo.tile([P, 1024], F32, tag="o2")
                nc.vector.tensor_tensor(os[:, 0:512], ps[:, 0:512],
                                        bo_bc[:, nch * 512:(nch + 1) * 512], op=ALU.add)
                nc.vector.tensor_tensor(os[:, 512:1024], ps[:, 512:1024],
                                        bo_bc[:, nch * 512:(nch + 1) * 512], op=ALU.add)
                nc.sync.dma_start(out[tb * P:(tb + 1) * P, nch * 512:(nch + 1) * 512],
                                  os[:, 0:512])
                nc.sync.dma_start(out[(tb + 1) * P:(tb + 2) * P, nch * 512:(nch + 1) * 512],
                                  os[:, 512:1024])


N_CORES = 8


_GRAPH_CACHE = {}


def build_graph():
    if "nc" in _GRAPH_CACHE:
        return _GRAPH_CACHE["nc"]
    nc = bacc.Bacc("TRN2", target_bir_lowering=False, debug=False,
                   num_devices=N_CORES)
    xT = nc.dram_tensor("xT", [D, NKV], BF16, kind="ExternalInput").ap()
    wqT = nc.dram_tensor("wqT", [D, D], BF16, kind="ExternalInput").ap()
    wkT = nc.dram_tensor("wkT", [D, D], BF16, kind="ExternalInput").ap()
    wvT = nc.dram_tensor("wvT", [D, D], BF16, kind="ExternalInput").ap()
    woT = nc.dram_tensor("woT", [D, D], BF16, kind="ExternalInput").ap()
    bq = nc.dram_tensor("bq", [P, CC], F32, kind="ExternalInput").ap()
    bk = nc.dram_tensor("bk", [P, CC], F32, kind="ExternalInput").ap()
    bv = nc.dram_tensor("bv", [P, CC], F32, kind="ExternalInput").ap()
    bo = nc.dram_tensor("bo", [1, D], F32, kind="ExternalInput").ap()
    out = nc.dram_tensor("out", [NQ, D], F32, kind="ExternalOutput").ap()
    with tile.TileContext(nc) as tc:
        attention_body(tc, out, xT, wqT, wkT, wvT, woT, bq, bk, bv, bo)
    nc.compile()
    _GRAPH_CACHE["nc"] = nc
    return nc


def make_in_maps(x, Wq, bq, Wk, bk, Wv, bv, Wo, bo):
    x = np.asarray(x, np.float32)
    shared = {
        "wqT": np.ascontiguousarray(np.asarray(Wq, np.float32).T).astype(BF),
        "wkT": np.ascontiguousarray(np.asarray(Wk, np.float32).T).astype(BF),
        "wvT": np.ascontiguousarray(np.asarray(Wv, np.float32).T).astype(BF),
        "woT": np.ascontiguousarray(np.asarray(Wo, np.float32).T).astype(BF),
        "bq": np.ascontiguousarray(np.asarray(bq, np.float32).reshape(CC, P).T),
        "bk": np.ascontiguousarray(np.asarray(bk, np.float32).reshape(CC, P).T),
        "bv": np.ascontiguousarray(np.asarray(bv, np.float32).reshape(CC, P).T),
        "bo": np.asarray(bo, np.float32).reshape(1, D),
    }
    in_maps = []
    for core in range(N_CORES):
        b, half = core // 2, core % 2
        xb = x[b]
        if half == 1:
            xb = np.concatenate([xb[NQ:], xb[:NQ]], axis=0)
        xT = np.ascontiguousarray(xb.T).astype(BF)
        in_maps.append({"xT": xT, **shared})
    return in_maps


def run(inputs, trace=False, **kw):
    nc = build_graph()
    in_maps = make_in_maps(**inputs)
    res = run_bass_kernel_spmd(nc, in_maps, list(range(N_CORES)), trace=trace, **kw)
    x = np.asarray(inputs["x"], np.float32)
    B, N, C = x.shape
    out = np.empty((B, N, C), np.float32)
    for core in range(N_CORES):
        b, half = core // 2, core % 2
        out[b, half * NQ:(half + 1) * NQ, :] = res.results[core]["out"]
    return out, res


def kernel(x, Wq, bq, Wk, bk, Wv, bv, Wo, bo):
    out, _ = run(dict(x=x, Wq=Wq, bq=bq, Wk=Wk, bk=bk, Wv=Wv, bv=bv, Wo=Wo, bo=bo))
    return out


# revision 14
# speedup vs baseline: 1.1813x; 1.0858x over previous
"""Fallback: original proven kernel + blocked xT DMA + q/k bias on DVE +
2-chunk score/AV batching. Original 65-col V slab and DMA-bcast eviction."""

from contextlib import ExitStack

import numpy as np
import ml_dtypes

import concourse.bass as bass
import concourse.bacc as bacc
import concourse.tile as tile
import concourse.mybir as mybir
from concourse.bass_utils import run_bass_kernel_spmd

F32 = mybir.dt.float32
BF16 = mybir.dt.bfloat16
AF = mybir.ActivationFunctionType
ALU = mybir.AluOpType
BF = ml_dtypes.bfloat16

P = 128
D = 1024
CC = 8
H = 16
DH = 64
NKV = 2048
NQ = 1024
TB = NKV // P
KC = NKV // P
SCALE = DH ** -0.5
VS = 2 * (8 * 65 + 64)


def bcast_row(nc, out_ap, src_row, n_part):
    ap0 = src_row.ap[0]
    free = src_row.ap[-1]
    src = bass.AP(src_row.tensor, src_row.offset, [ap0, [0, n_part], free])
    nc.sync.dma_start(out_ap, src)


def attention_body(tc, out, xT, wqT, wkT, wvT, woT, bq, bk, bv, bo):
    nc = tc.nc
    with ExitStack() as ctx:
        const = ctx.enter_context(tc.tile_pool(name="const", bufs=1))
        qkv = ctx.enter_context(tc.tile_pool(name="qkv", bufs=1))
        xw = ctx.enter_context(tc.tile_pool(name="xw", bufs=1))
        wst = ctx.enter_context(tc.tile_pool(name="wst", bufs=2))
        wib = ctx.enter_context(tc.tile_pool(name="wib", bufs=3))
        ee = ctx.enter_context(tc.tile_pool(name="ee", bufs=3))
        rc = ctx.enter_context(tc.tile_pool(name="rc", bufs=1))
        fo = ctx.enter_context(tc.tile_pool(name="fo", bufs=2))
        sp = ctx.enter_context(tc.tile_pool(name="sp", bufs=2, space="PSUM"))
        ao = ctx.enter_context(tc.tile_pool(name="ao", bufs=1, space="PSUM"))
        pj = ctx.enter_context(tc.tile_pool(name="pj", bufs=2, space="PSUM"))

        bq_sb = const.tile([P, CC], F32)
        bk_sb = const.tile([P, CC], F32)
        bv_sb = const.tile([P, CC], F32)
        bo_sb = const.tile([1, D], F32)
        nc.sync.dma_start(bq_sb[:, :], bq[:, :])
        nc.sync.dma_start(bk_sb[:, :], bk[:, :])
        nc.sync.dma_start(bv_sb[:, :], bv[:, :])
        nc.sync.dma_start(bo_sb[:, :], bo[:, :])
        onesf = const.tile([1, P], F32)
        nc.vector.memset(onesf[:, :], 1.0)
        bo_bc = const.tile([P, D], BF16)

        qT_sb = qkv.tile([P, CC * NQ], BF16)
        kT_sb = qkv.tile([P, CC * NKV], BF16)
        v_sb = qkv.tile([P, TB * VS], BF16)
        yT_sb = qkv.tile([P, CC * NQ], BF16)

        def load_w(wT_dram):
            w_sb = wst.tile([P, CC * D], BF16, tag="w")
            for cc in range(CC):
                nc.sync.dma_start(w_sb[:, cc * D:(cc + 1) * D], wT_dram[cc * P:(cc + 1) * P, :])
            return w_sb

        wv_sb = load_w(wvT)

        xT_sb = xw.tile([P, CC * NKV], BF16)
        for blk in range(4):
            for cc in range(CC):
                nc.sync.dma_start(
                    xT_sb[:, cc * NKV + blk * 512: cc * NKV + (blk + 1) * 512],
                    xT[cc * P:(cc + 1) * P, blk * 512:(blk + 1) * 512])

        def load_w_ib(wT_dram, ib):
            w_sb = wib.tile([P, CC * P], BF16, tag="wib")
            for cc in range(CC):
                nc.sync.dma_start(
                    w_sb[:, cc * P:(cc + 1) * P],
                    wT_dram[cc * P:(cc + 1) * P, ib * P:(ib + 1) * P])
            return w_sb

        v4 = v_sb.rearrange("p (t g s) -> p t g s", t=TB, g=2)
        nc.vector.memset(v4[:, :, :, 8 * 65:], 0.0)
        v5 = v4[:, :, :, 0:8 * 65].rearrange("p t g (h c) -> p t g h c", c=65)
        nc.vector.memset(v5[:, :, :, :, 64:65], 1.0)

        def v_proj(tb, nch):
            ps = pj.tile([P, 512], F32, tag="ps")
            for cc in range(CC):
                nc.tensor.matmul(
                    ps[:, :],
                    xT_sb[:, cc * NKV + tb * P: cc * NKV + (tb + 1) * P],
                    wv_sb[:, cc * D + nch * 512: cc * D + nch * 512 + 512],
                    start=(cc == 0), stop=(cc == CC - 1))
            vsrc = ps.rearrange("p (h c) -> p h c", c=64)
            base = tb * VS + nch * (8 * 65 + 64)
            dst = v_sb[:, base: base + 8 * 65].rearrange(
                "p (h c) -> p h c", c=65)[:, :, 0:64]
            nc.vector.tensor_copy(dst, vsrc)

        for tb in range(TB):
            v_proj(tb, 0)

        for ch in range(2):
            ps = pj.tile([P, 512], F32, tag="ps")
            nc.tensor.matmul(ps[:, :], onesf[:, :], bo_sb[:, ch * 512:(ch + 1) * 512],
                             start=True, stop=True)
            nc.vector.tensor_copy(bo_bc[:, ch * 512:(ch + 1) * 512], ps[:, :])

        def q_proj(ib, w_sb=None):
            if w_sb is None:
                w_sb = load_w_ib(wqT, ib)
            for t in range(NQ // 512):
                ps = pj.tile([P, 512], F32, tag="ps")
                for cc in range(CC):
                    nc.tensor.matmul(
                        ps[:, :],
                        w_sb[:, cc * P:(cc + 1) * P],
                        xT_sb[:, cc * NKV + t * 512: cc * NKV + t * 512 + 512],
                        start=(cc == 0), stop=(cc == CC - 1))
                nc.vector.tensor_scalar(
                    qT_sb[:, ib * NQ + t * 512: ib * NQ + t * 512 + 512],
                    ps[:, :], bq_sb[:, ib:ib + 1], None, op0=ALU.add)

        def k_proj(ib, w_sb=None):
            if w_sb is None:
                w_sb = load_w_ib(wkT, ib)
            for t in range(NKV // 512):
                ps = pj.tile([P, 512], F32, tag="ps")
                for cc in range(CC):
                    nc.tensor.matmul(
                        ps[:, :],
                        w_sb[:, cc * P:(cc + 1) * P],
                        xT_sb[:, cc * NKV + t * 512: cc * NKV + t * 512 + 512],
                        start=(cc == 0), stop=(cc == CC - 1))
                nc.vector.tensor_scalar(
                    kT_sb[:, ib * NKV + t * 512: ib * NKV + t * 512 + 512],
                    ps[:, :], bk_sb[:, ib:ib + 1], None, op0=ALU.add)

        q_proj(0)
        k_proj(0)

        wo_sb = None

        def fin_proj(tb):
            for nch in range(2):
                ps = pj.tile([P, 512], F32, tag="ps")
                for cc in range(CC):
                    nc.tensor.matmul(
                        ps[:, :],
                        yT_sb[:, cc * NQ + tb * P: cc * NQ + (tb + 1) * P],
                        wo_sb[:, cc * D + nch * 512: cc * D + nch * 512 + 512],
                        start=(cc == 0), stop=(cc == CC - 1))
                os = fo.tile([P, 512], F32, tag="o")
                nc.vector.tensor_tensor(os[:, :], ps[:, :],
                                        bo_bc[:, nch * 512:(nch + 1) * 512], op=ALU.add)
                nc.sync.dma_start(out[tb * P:(tb + 1) * P, nch * 512:(nch + 1) * 512],
                                  os[:, :])

        for pr in range(CC):
            hA, hB = 2 * pr, 2 * pr + 1
            for qb in range(NQ // 512):
                qc = qb * 512
                oA = ao.tile([P, 512], F32, tag="oA")
                oB = ao.tile([P, 512], F32, tag="oB")
                vbA = (hA // 8) * 584 + (hA % 8) * 65
                vbB = (hB // 8) * 584 + (hB % 8) * 65
                for kc2 in range(KC // 2):
                    pair = []
                    for kc in (2 * kc2, 2 * kc2 + 1):
                        s2 = sp.tile([P, 1024], F32, tag="s")
                        nc.tensor.matmul(
                            s2[:, 0:512],
                            kT_sb[0:64, pr * NKV + kc * P: pr * NKV + (kc + 1) * P],
                            qT_sb[0:64, pr * NQ + qc: pr * NQ + qc + 512],
                            start=True, stop=True)
                        nc.tensor.matmul(
                            s2[:, 512:1024],
                            kT_sb[64:128, pr * NKV + kc * P: pr * NKV + (kc + 1) * P],
                            qT_sb[64:128, pr * NQ + qc: pr * NQ + qc + 512],
                            start=True, stop=True)
                        e2 = ee.tile([P, 1024], BF16, tag="e")
                        nc.scalar.activation(e2[:, :], s2[:, :], AF.Exp, scale=SCALE)
                        pair.append((kc, e2))
                    for kc, e2 in pair:
                        nc.tensor.matmul(
                            oA[:, :],
                            v_sb[:, kc * VS + vbA: kc * VS + vbA + 128],
                            e2[:, 0:512],
                            start=(kc == 0), stop=(kc == KC - 1))
                        nc.tensor.matmul(
                            oB[:, :],
                            v_sb[:, kc * VS + vbB: kc * VS + vbB + 128],
                            e2[:, 512:1024],
                            start=(kc == 0), stop=(kc == KC - 1))
                yA = yT_sb[0:64, pr * NQ + qc: pr * NQ + qc + 512]
                yB = yT_sb[64:128, pr * NQ + qc: pr * NQ + qc + 512]
                den2 = rc.tile([1, 1024], F32, tag="d")
                nc.vector.tensor_copy(den2[0:1, 0:512], oA[64:65, :])
                nc.vector.tensor_copy(yA, oA[0:64, :])
                nc.vector.tensor_copy(den2[0:1, 512:1024], oB[64:65, :])
                nc.vector.tensor_copy(yB, oB[0:64, :])
                rec2 = rc.tile([1, 1024], F32, tag="rf")
                nc.vector.reciprocal_approx_fast(rec2[0:1, :], den2[0:1, :])
                bc2 = rc.tile([P, 512], F32, tag="bc")
                bcast_row(nc, bc2[0:64, :], rec2[0:1, 0:512], 64)
                bcast_row(nc, bc2[64:128, :], rec2[0:1, 512:1024], 64)
                nc.vector.tensor_tensor(yA, yA, bc2[0:64, :], op=ALU.mult)
                nc.vector.tensor_scalar(yA, yA, bv_sb[0:64, pr:pr + 1], None, op0=ALU.add)
                nc.vector.tensor_tensor(yB, yB, bc2[64:128, :], op=ALU.mult)
                nc.vector.tensor_scalar(yB, yB, bv_sb[64:128, pr:pr + 1], None, op0=ALU.add)

                if pr < 4:
                    v_proj(4 * pr + 2 * qb, 1)
                    v_proj(4 * pr + 2 * qb + 1, 1)
                if qb == 0 and pr + 1 < CC:
                    wq_next = load_w_ib(wqT, pr + 1)
                    wk_next = load_w_ib(wkT, pr + 1)
                    q_proj(pr + 1, wq_next)
                elif qb == 1 and pr + 1 < CC:
                    k_proj(pr + 1, wk_next)
                if pr == 5 and qb == 1:
                    wo_sb = load_w(woT)
                if pr == CC - 1 and qb == 1:
                    for ftb in range(NQ // P // 2):
                        fin_proj(ftb)

        for tb in range(NQ // P // 2, NQ // P, 2):
            for nch in range(2):
                ps = sp.tile([P, 1024], F32, tag="s")
                for j in range(2):
                    for cc in range(CC):
                        nc.tensor.matmul(
                            ps[:, j * 512:(j + 1) * 512],
                            yT_sb[:, cc * NQ + (tb + j) * P: cc * NQ + (tb + j + 1) * P],
                            wo_sb[:, cc * D + nch * 512: cc * D + nch * 512 + 512],
                            start=(cc == 0), stop=(cc == CC - 1))
                os = fo.tile([P, 1024], F32, tag="o2")
                nc.vector.tensor_tensor(os[:, 0:512], ps[:, 0:512],
                                        bo_bc[:, nch * 512:(nch + 1) * 512], op=ALU.add)
                nc.vector.tensor_tensor(os[:, 512:1024], ps[:, 512:1024],
                                        bo_bc[:, nch * 512:(nch + 1) * 512], op=ALU.add)
                nc.sync.dma_start(out[tb * P:(tb + 1) * P, nch * 512:(nch + 1) * 512],
                                  os[:, 0:512])
                nc.sync.dma_start(out[(tb + 1) * P:(tb + 2) * P, nch * 512:(nch + 1) * 512],
                                  os[:, 512:1024])


N_CORES = 8


_GRAPH_CACHE = {}


def build_graph():
    if "nc" in _GRAPH_CACHE:
        return _GRAPH_CACHE["nc"]
    nc = bacc.Bacc("TRN2", target_bir_lowering=False, debug=False,
                   num_devices=N_CORES)
    xT = nc.dram_tensor("xT", [D, NKV], BF16, kind="ExternalInput").ap()
    wqT = nc.dram_tensor("wqT", [D, D], BF16, kind="ExternalInput").ap()
    wkT = nc.dram_tensor("wkT", [D, D], BF16, kind="ExternalInput").ap()
    wvT = nc.dram_tensor("wvT", [D, D], BF16, kind="ExternalInput").ap()
    woT = nc.dram_tensor("woT", [D, D], BF16, kind="ExternalInput").ap()
    bq = nc.dram_tensor("bq", [P, CC], F32, kind="ExternalInput").ap()
    bk = nc.dram_tensor("bk", [P, CC], F32, kind="ExternalInput").ap()
    bv = nc.dram_tensor("bv", [P, CC], F32, kind="ExternalInput").ap()
    bo = nc.dram_tensor("bo", [1, D], F32, kind="ExternalInput").ap()
    out = nc.dram_tensor("out", [NQ, D], F32, kind="ExternalOutput").ap()
    with tile.TileContext(nc) as tc:
        attention_body(tc, out, xT, wqT, wkT, wvT, woT, bq, bk, bv, bo)
    nc.compile()
    _GRAPH_CACHE["nc"] = nc
    return nc


def make_in_maps(x, Wq, bq, Wk, bk, Wv, bv, Wo, bo):
    x = np.asarray(x, np.float32)
    shared = {
        "wqT": np.ascontiguousarray(np.asarray(Wq, np.float32).T).astype(BF),
        "wkT": np.ascontiguousarray(np.asarray(Wk, np.float32).T).astype(BF),
        "wvT": np.ascontiguousarray(np.asarray(Wv, np.float32).T).astype(BF),
        "woT": np.ascontiguousarray(np.asarray(Wo, np.float32).T).astype(BF),
        "bq": np.ascontiguousarray(np.asarray(bq, np.float32).reshape(CC, P).T),
        "bk": np.ascontiguousarray(np.asarray(bk, np.float32).reshape(CC, P).T),
        "bv": np.ascontiguousarray(np.asarray(bv, np.float32).reshape(CC, P).T),
        "bo": np.asarray(bo, np.float32).reshape(1, D),
    }
    in_maps = []
    for core in range(N_CORES):
        b, half = core // 2, core % 2
        xb = x[b]
        if half == 1:
            xb = np.concatenate([xb[NQ:], xb[:NQ]], axis=0)
        xT = np.ascontiguousarray(xb.T).astype(BF)
        in_maps.append({"xT": xT, **shared})
    return in_maps


def run(inputs, trace=False, **kw):
    nc = build_graph()
    in_maps = make_in_maps(**inputs)
    res = run_bass_kernel_spmd(nc, in_maps, list(range(N_CORES)), trace=trace, **kw)
    x = np.asarray(inputs["x"], np.float32)
    B, N, C = x.shape
    out = np.empty((B, N, C), np.float32)
    for core in range(N_CORES):
        b, half = core // 2, core % 2
        out[b, half * NQ:(half + 1) * NQ, :] = res.results[core]["out"]
    return out, res


def kernel(x, Wq, bq, Wk, bk, Wv, bv, Wo, bo):
    out, _ = run(dict(x=x, Wq=Wq, bq=bq, Wk=Wk, bk=bk, Wv=Wv, bv=bv, Wo=Wo, bo=bo))
    return out
